# revision 1
# baseline (speedup 1.0000x reference)
import sys

sys.path.insert(0, "/opt/trn_rl_repo")

import numpy as np

import concourse.bacc as bacc
import concourse.bass as bass
import concourse.mybir as mybir
import concourse.tile as tile
from concourse.bass_utils import run_bass_kernel_spmd

# Problem shapes (hardcoded per contract)
B = 4
NQ = 2048
NR = 16384
D = 64
K = 16

NCORES = 8
QPC = NQ // 2          # queries per core (each batch split across 2 cores)
NCHUNK = QPC // 128    # query chunks of 128 per core
MMN = 512              # matmul free dim (one PSUM bank of fp32)
GRP = 1024             # candidate block width (2 PSUM banks); top-8 per group
NGRP = NR // GRP       # 16 groups
NCAND = NGRP * 8       # 128 candidates per row

_prog_cache = {}


def _build_program(reps: int = 1):
    if reps in _prog_cache:
        return _prog_cache[reps]

    f32 = mybir.dt.float32
    u32 = mybir.dt.uint32

    nc = bacc.Bacc("TRN2", target_bir_lowering=False, debug=False, num_devices=NCORES)

    # lhsT rows 0..63 = 2*q^T, row 64 = 1.0, row 65 = q2  -> psum = 2qr - r2 - q2 = -d2
    lhs_d = nc.dram_tensor("lhs", [66, QPC], f32, kind="ExternalInput")
    rhs_d = nc.dram_tensor("rhs", [66, NR], f32, kind="ExternalInput")

    outD_d = nc.dram_tensor("outD", [QPC, K], f32, kind="ExternalOutput")
    outP_d = nc.dram_tensor("outP", [QPC, K], u32, kind="ExternalOutput")
    outCI_d = nc.dram_tensor("outCI", [QPC, NCAND], u32, kind="ExternalOutput")

    with tile.TileContext(nc) as tc:
        with (
            tc.tile_pool(name="consts", bufs=1) as cpool,
            tc.tile_pool(name="psum", bufs=8, space="PSUM") as ppool,
            tc.tile_pool(name="stage", bufs=6) as spool,
            tc.tile_pool(name="cands", bufs=3) as candpool,
            tc.tile_pool(name="merge", bufs=2) as mpool,
        ):
            lhs_t = cpool.tile([66, QPC], f32)
            nc.sync.dma_start(lhs_t[:], lhs_d.ap())
            rhs_t = cpool.tile([66, NR], f32)
            nc.sync.dma_start(rhs_t[:], rhs_d.ap())

            for rep in range(reps):
              for c in range(NCHUNK):
                cands_v = candpool.tile([128, NCAND], f32, tag="cv")
                cands_i = candpool.tile([128, NCAND], u32, tag="ci")
                for g in range(NGRP):
                    st = spool.tile([128, GRP], f32, tag="st")
                    for h in range(GRP // MMN):
                        ps = ppool.tile([128, MMN], f32, tag="ps")
                        nc.tensor.matmul(
                            ps[:],
                            lhs_t[:, c * 128:(c + 1) * 128],
                            rhs_t[:, g * GRP + h * MMN:g * GRP + (h + 1) * MMN],
                            start=True,
                            stop=True,
                        )
                        nc.scalar.copy(st[:, h * MMN:(h + 1) * MMN], ps[:])
                    s = g * 8
                    nc.vector.max(cands_v[:, s:s + 8], st[:])
                    nc.vector.max_index(cands_i[:, s:s + 8], cands_v[:, s:s + 8], st[:])

                # merge candidates -> top-16 (values + candidate slots)
                v16 = mpool.tile([128, K], f32, tag="v16")
                p16 = mpool.tile([128, K], u32, tag="p16")
                mr = mpool.tile([128, NCAND], f32, tag="mr")
                nc.vector.max(v16[:, 0:8], cands_v[:])
                nc.vector.max_index(p16[:, 0:8], v16[:, 0:8], cands_v[:])
                nc.vector.match_replace(mr[:], v16[:, 0:8], cands_v[:], -1e30)
                nc.vector.max(v16[:, 8:16], mr[:])
                nc.vector.max_index(p16[:, 8:16], v16[:, 8:16], mr[:])

                # D = sqrt(relu(-v16))
                dsq = mpool.tile([128, K], f32, tag="dsq")
                d16 = mpool.tile([128, K], f32, tag="d16")
                nc.scalar.activation(
                    dsq[:], v16[:], mybir.ActivationFunctionType.Relu, scale=-1.0
                )
                nc.scalar.activation(d16[:], dsq[:], mybir.ActivationFunctionType.Sqrt)

                r0, r1 = c * 128, (c + 1) * 128
                nc.sync.dma_start(outD_d.ap()[r0:r1, :], d16[:])
                nc.sync.dma_start(outP_d.ap()[r0:r1, :], p16[:])
                nc.sync.dma_start(outCI_d.ap()[r0:r1, :], cands_i[:])

    nc.compile()
    _prog_cache[reps] = nc
    return nc


def kernel(ref: np.ndarray, query: np.ndarray):
    ref = np.asarray(ref, dtype=np.float32)
    query = np.asarray(query, dtype=np.float32)

    # host-side operand prep (layout + norms)
    r2 = np.sum(ref * ref, axis=-1)                      # [B, NR]
    q2 = np.sum(query * query, axis=-1)                  # [B, NQ]
    refT = np.ascontiguousarray(ref.transpose(0, 2, 1))  # [B, D, NR]
    qT = np.ascontiguousarray(query.transpose(0, 2, 1))  # [B, D, NQ]

    nc = _build_program()

    in_maps = []
    for core in range(NCORES):
        b, h = core // 2, core % 2
        lhs = np.empty((66, QPC), dtype=np.float32)
        lhs[0:D, :] = 2.0 * qT[b][:, h * QPC:(h + 1) * QPC]
        lhs[D, :] = 1.0
        lhs[D + 1, :] = q2[b, h * QPC:(h + 1) * QPC]
        rhs = np.empty((66, NR), dtype=np.float32)
        rhs[0:D, :] = refT[b]
        rhs[D, :] = -r2[b]
        rhs[D + 1, :] = -1.0
        in_maps.append({"lhs": lhs, "rhs": rhs})

    res = run_bass_kernel_spmd(nc, in_maps, core_ids=list(range(NCORES)))

    Dout = np.empty((B, NQ, K), dtype=np.float32)
    Iout = np.empty((B, NQ, K), dtype=np.int64)
    rows = np.arange(QPC)[:, None]
    for core in range(NCORES):
        b, h = core // 2, core % 2
        r = res.results[core]
        d16 = r["outD"]                      # [QPC, K] f32
        p16 = r["outP"].astype(np.int64)     # [QPC, K] candidate slots
        ci = r["outCI"].astype(np.int64)     # [QPC, NCAND] local idx in group
        gi = ci[rows, p16] + GRP * (p16 >> 3)
        Dout[b, h * QPC:(h + 1) * QPC] = d16
        Iout[b, h * QPC:(h + 1) * QPC] = gi
    return (Dout, Iout)



# revision 2
# speedup vs baseline: 1.8545x; 1.8545x over previous
import sys

sys.path.insert(0, "/opt/trn_rl_repo")

import numpy as np

import concourse.bacc as bacc
import concourse.bass as bass
import concourse.mybir as mybir
import concourse.tile as tile
from concourse.bass_utils import run_bass_kernel_spmd

# Problem shapes (hardcoded per contract)
B = 4
NQ = 2048
NR = 16384
D = 64
K = 16

NCORES = 8
QPC = NQ // 2          # queries per core (each batch split across 2 cores)
NCHUNK = QPC // 128    # query chunks of 128 per core
MMN = 512              # matmul free dim (one PSUM bank of fp32)
PAIR = 2048            # refs per staging tile (2 candidate groups, 4 PSUM banks)
NPAIR = NR // PAIR     # 8
GRP = 1024             # candidate block width; top-8 per group
NGRP = NR // GRP       # 16
NCAND = NGRP * 8       # 128 candidates per row

_prog_cache = {}


def _build_program(reps: int = 1):
    if reps in _prog_cache:
        return _prog_cache[reps]

    f32 = mybir.dt.float32
    f32r = mybir.dt.float32r
    f16 = mybir.dt.float16
    u32 = mybir.dt.uint32

    nc = bacc.Bacc("TRN2", target_bir_lowering=False, debug=False, num_devices=NCORES)

    # lhsT rows 0..63 = 2*q^T, row 64 = 1.0, row 65 = q2  -> psum = 2qr - r2 - q2 = -d2
    lhs_d = nc.dram_tensor("lhs", [66, QPC], f32r, kind="ExternalInput")
    rhs_d = nc.dram_tensor("rhs", [66, NR], f32r, kind="ExternalInput")
    # global ref index per composite low-half: iota[0, j] = j
    iota_d = nc.dram_tensor("iota", [1, NR], u32, kind="ExternalInput")

    # composite top-16 per query: fp16(-d2) in high 16 bits, ref idx in low 14
    outV_d = nc.dram_tensor("outV", [QPC, K], u32, kind="ExternalOutput")

    with tile.TileContext(nc) as tc:
        with (
            tc.tile_pool(name="consts", bufs=1) as cpool,
            tc.tile_pool(name="psum", bufs=2, space="PSUM") as ppool,
            tc.tile_pool(name="merge", bufs=2) as mpool,
        ):
            lhs_t = cpool.tile([66, QPC], f32r)
            nc.sync.dma_start(lhs_t[:], lhs_d.ap())
            rhs_t = cpool.tile([66, NR], f32r)
            nc.sync.dma_start(rhs_t[:], rhs_d.ap())

            # persistent composite staging: one slot per ref-pair-block; low
            # halves carry the global ref index and are written exactly once
            stages = []
            for p in range(NPAIR):
                st = cpool.tile([128, PAIR], f32, name=f"stage{p}")
                nc.sync.dma_start(
                    st.bitcast(u32)[:],
                    iota_d.ap()[0:1, p * PAIR:(p + 1) * PAIR].partition_broadcast(128),
                )
                stages.append(st)

            for rep in range(reps):
              for c in range(NCHUNK):
                cands = mpool.tile([128, NCAND], f32, tag="cands", bufs=1)
                for p in range(NPAIR):
                    ps = ppool.tile([128, PAIR], f32, tag="ps")
                    for h in range(PAIR // MMN):
                        nc.tensor.matmul(
                            ps[:, h * MMN:(h + 1) * MMN],
                            lhs_t[:, c * 128:(c + 1) * 128],
                            rhs_t[:, p * PAIR + h * MMN:p * PAIR + (h + 1) * MMN],
                            start=True,
                            stop=True,
                        )
                    # -d2 as fp16 into composite high halves (strided write)
                    nc.scalar.activation(
                        stages[p].bitcast(f16)[:, 1::2],
                        ps[:],
                        mybir.ActivationFunctionType.Copy,
                    )
                    s = p * 16
                    nc.vector.max(cands[:, s:s + 8], stages[p][:, 0:GRP])
                    nc.vector.max(cands[:, s + 8:s + 16], stages[p][:, GRP:PAIR])

                # merge 128 candidate composites -> top-16 (desc = d2 asc)
                v16 = mpool.tile([128, K], f32, tag="v16")
                mr = mpool.tile([128, NCAND], f32, tag="mr")
                nc.vector.max(v16[:, 0:8], cands[:])
                nc.vector.match_replace(mr[:], v16[:, 0:8], cands[:], -1e30)
                nc.vector.max(v16[:, 8:16], mr[:])

                r0, r1 = c * 128, (c + 1) * 128
                nc.sync.dma_start(outV_d.ap()[r0:r1, :], v16.bitcast(u32)[:])

    nc.compile()
    _prog_cache[reps] = nc
    return nc


def kernel(ref: np.ndarray, query: np.ndarray):
    ref = np.asarray(ref, dtype=np.float32)
    query = np.asarray(query, dtype=np.float32)

    # host-side operand prep (layout + norms)
    r2 = np.sum(ref * ref, axis=-1)                      # [B, NR]
    q2 = np.sum(query * query, axis=-1)                  # [B, NQ]
    refT = np.ascontiguousarray(ref.transpose(0, 2, 1))  # [B, D, NR]
    qT = np.ascontiguousarray(query.transpose(0, 2, 1))  # [B, D, NQ]
    iota = np.arange(NR, dtype=np.uint32)[None, :]

    nc = _build_program()

    in_maps = []
    for core in range(NCORES):
        b, h = core // 2, core % 2
        lhs = np.empty((66, QPC), dtype=np.float32)
        lhs[0:D, :] = 2.0 * qT[b][:, h * QPC:(h + 1) * QPC]
        lhs[D, :] = 1.0
        lhs[D + 1, :] = q2[b, h * QPC:(h + 1) * QPC]
        rhs = np.empty((66, NR), dtype=np.float32)
        rhs[0:D, :] = refT[b]
        rhs[D, :] = -r2[b]
        rhs[D + 1, :] = -1.0
        in_maps.append({"lhs": lhs, "rhs": rhs, "iota": iota})

    res = run_bass_kernel_spmd(nc, in_maps, core_ids=list(range(NCORES)))

    Dout = np.empty((B, NQ, K), dtype=np.float32)
    Iout = np.empty((B, NQ, K), dtype=np.int64)
    for core in range(NCORES):
        b, h = core // 2, core % 2
        comp = res.results[core]["outV"].astype(np.uint32)   # [QPC, K]
        idx = (comp & 0x3FFF).astype(np.int64)
        val = (comp >> 16).astype(np.uint16).view(np.float16).astype(np.float32)
        Dout[b, h * QPC:(h + 1) * QPC] = np.sqrt(np.maximum(0.0, -val))
        Iout[b, h * QPC:(h + 1) * QPC] = idx
    return (Dout, Iout)


# revision 6
# speedup vs baseline: 1.8951x; 1.0219x over previous
import sys

sys.path.insert(0, "/opt/trn_rl_repo")

import numpy as np

import concourse.bacc as bacc
import concourse.bass as bass
import concourse.mybir as mybir
import concourse.tile as tile
from concourse.bass_utils import run_bass_kernel_spmd

# Problem shapes (hardcoded per contract)
B = 4
NQ = 2048
NR = 16384
D = 64
K = 16

NCORES = 8
QPC = NQ // 2          # queries per core (each batch split across 2 cores)
NCHUNK = QPC // 128    # query chunks of 128 per core
MMN = 512              # matmul free dim (one PSUM bank of fp32)
PAIR = 2048            # refs per staging tile (2 candidate groups, 4 PSUM banks)
NPAIR = NR // PAIR     # 8
GRP = 1024             # candidate block width; top-8 per group
NGRP = NR // GRP       # 16
NCAND = NGRP * 8       # 128 candidates per row

_prog_cache = {}


def _build_program(reps: int = 1):
    if reps in _prog_cache:
        return _prog_cache[reps]

    f32 = mybir.dt.float32
    f32r = mybir.dt.float32r
    f16 = mybir.dt.float16
    u32 = mybir.dt.uint32

    nc = bacc.Bacc("TRN2", target_bir_lowering=False, debug=False, num_devices=NCORES)

    # lhsT rows 0..63 = 2*q^T, row 64 = 1.0, row 65 = q2  -> psum = 2qr - r2 - q2 = -d2
    lhs_d = nc.dram_tensor("lhs", [66, QPC], f32r, kind="ExternalInput")
    rhs_d = nc.dram_tensor("rhs", [66, NR], f32r, kind="ExternalInput")
    # global ref index per composite low-half: iota[0, j] = j
    iota_d = nc.dram_tensor("iota", [1, NR], u32, kind="ExternalInput")

    # composite top-16 per query: fp16(-d2) in high 16 bits, ref idx in low 14
    outV_d = nc.dram_tensor("outV", [QPC, K], u32, kind="ExternalOutput")

    with tile.TileContext(nc) as tc:
        with (
            tc.tile_pool(name="consts", bufs=1) as cpool,
            tc.tile_pool(name="psum", bufs=2, space="PSUM") as ppool,
            tc.tile_pool(name="merge", bufs=2) as mpool,
        ):
            lhs_t = cpool.tile([66, QPC], f32r)
            nc.sync.dma_start(lhs_t[:], lhs_d.ap())
            rhs_t = cpool.tile([66, NR], f32r)

            # persistent composite staging: one slot per ref-pair-block; low
            # halves carry the global ref index and are written exactly once.
            # Interleave per-pair rhs/iota DMAs so pair p can start computing
            # as soon as its own slices land.
            stages = []
            for p in range(NPAIR):
                c0, c1 = p * PAIR, (p + 1) * PAIR
                nc.sync.dma_start(rhs_t[:, c0:c1], rhs_d.ap()[:, c0:c1])
                st = cpool.tile([128, PAIR], f32, name=f"stage{p}")
                nc.sync.dma_start(
                    st.bitcast(u32)[:],
                    iota_d.ap()[0:1, c0:c1].partition_broadcast(128),
                )
                stages.append(st)

            for rep in range(reps):
              for c in range(NCHUNK):
                cands = mpool.tile([128, NCAND], f32, tag="cands", bufs=2)
                for p in range(NPAIR):
                    ps = ppool.tile([128, PAIR], f32, tag="ps")
                    for h in range(PAIR // MMN):
                        nc.tensor.matmul(
                            ps[:, h * MMN:(h + 1) * MMN],
                            lhs_t[:, c * 128:(c + 1) * 128],
                            rhs_t[:, p * PAIR + h * MMN:p * PAIR + (h + 1) * MMN],
                            start=True,
                            stop=True,
                        )
                    # -d2 as fp16 into composite high halves (strided write)
                    nc.scalar.activation(
                        stages[p].bitcast(f16)[:, 1::2],
                        ps[:],
                        mybir.ActivationFunctionType.Copy,
                    )
                    s = p * 16
                    nc.vector.max(cands[:, s:s + 8], stages[p][:, 0:GRP])
                    nc.vector.max(cands[:, s + 8:s + 16], stages[p][:, GRP:PAIR])

                # merge 128 candidate composites -> top-16 (desc = d2 asc)
                v16 = mpool.tile([128, K], f32, tag="v16")
                mr = mpool.tile([128, NCAND], f32, tag="mr")
                nc.vector.max(v16[:, 0:8], cands[:])
                nc.vector.match_replace(mr[:], v16[:, 0:8], cands[:], -1e30)
                nc.vector.max(v16[:, 8:16], mr[:])

                r0, r1 = c * 128, (c + 1) * 128
                nc.sync.dma_start(outV_d.ap()[r0:r1, :], v16.bitcast(u32)[:])

    nc.compile()
    _prog_cache[reps] = nc
    return nc


def kernel(ref: np.ndarray, query: np.ndarray):
    ref = np.asarray(ref, dtype=np.float32)
    query = np.asarray(query, dtype=np.float32)

    # host-side operand prep (layout + norms)
    r2 = np.sum(ref * ref, axis=-1)                      # [B, NR]
    q2 = np.sum(query * query, axis=-1)                  # [B, NQ]
    refT = np.ascontiguousarray(ref.transpose(0, 2, 1))  # [B, D, NR]
    qT = np.ascontiguousarray(query.transpose(0, 2, 1))  # [B, D, NQ]
    iota = np.arange(NR, dtype=np.uint32)[None, :]

    nc = _build_program()

    in_maps = []
    for core in range(NCORES):
        b, h = core // 2, core % 2
        lhs = np.empty((66, QPC), dtype=np.float32)
        lhs[0:D, :] = 2.0 * qT[b][:, h * QPC:(h + 1) * QPC]
        lhs[D, :] = 1.0
        lhs[D + 1, :] = q2[b, h * QPC:(h + 1) * QPC]
        rhs = np.empty((66, NR), dtype=np.float32)
        rhs[0:D, :] = refT[b]
        rhs[D, :] = -r2[b]
        rhs[D + 1, :] = -1.0
        in_maps.append({"lhs": lhs, "rhs": rhs, "iota": iota})

    res = run_bass_kernel_spmd(nc, in_maps, core_ids=list(range(NCORES)))

    Dout = np.empty((B, NQ, K), dtype=np.float32)
    Iout = np.empty((B, NQ, K), dtype=np.int64)
    rows = np.arange(QPC)[:, None]
    for core in range(NCORES):
        b, h = core // 2, core % 2
        comp = res.results[core]["outV"].astype(np.uint32)   # [QPC, K]
        idx = (comp & 0x3FFF).astype(np.int64)
        # exact rescore of the 16 device-selected candidates (fixes
        # quantization-induced order swaps among near-ties)
        qs = query[b, h * QPC:(h + 1) * QPC]                 # [QPC, D]
        cand = ref[b][idx]                                   # [QPC, K, D]
        d2 = np.maximum(0.0, np.sum((cand - qs[:, None, :]) ** 2, axis=-1))
        # sort by (d2, idx): exact ties keep smaller index first
        perm = np.lexsort((idx, d2), axis=1)
        Dout[b, h * QPC:(h + 1) * QPC] = np.sqrt(d2[rows, perm])
        Iout[b, h * QPC:(h + 1) * QPC] = idx[rows, perm]
    return (Dout, Iout)


# revision 9
# speedup vs baseline: 1.9081x; 1.0069x over previous
import sys

sys.path.insert(0, "/opt/trn_rl_repo")

import numpy as np

import concourse.bacc as bacc
import concourse.bass as bass
import concourse.mybir as mybir
import concourse.tile as tile
from concourse.bass_utils import run_bass_kernel_spmd

# Problem shapes (hardcoded per contract)
B = 4
NQ = 2048
NR = 16384
D = 64
K = 16

NCORES = 8
QPC = NQ // 2          # queries per core (each batch split across 2 cores)
NCHUNK = QPC // 128    # query chunks of 128 per core
MMN = 512              # matmul free dim (one PSUM bank of fp32)
PAIR = 2048            # refs per staging tile (2 candidate groups, 4 PSUM banks)
NPAIR = NR // PAIR     # 8
NCAND = NPAIR * 8      # 64 candidates per row (top-8 per 2048-block)

_prog_cache = {}


def _build_program(reps: int = 1):
    if reps in _prog_cache:
        return _prog_cache[reps]

    f32 = mybir.dt.float32
    f32r = mybir.dt.float32r
    f16 = mybir.dt.float16
    u32 = mybir.dt.uint32

    nc = bacc.Bacc("TRN2", target_bir_lowering=False, debug=False, num_devices=NCORES)

    # lhsT rows 0..63 = 2*q^T, row 64 = 1.0, row 65 = q2  -> psum = 2qr - r2 - q2 = -d2
    lhs_d = nc.dram_tensor("lhs", [66, QPC], f32r, kind="ExternalInput")
    rhs_d = nc.dram_tensor("rhs", [66, NR], f32r, kind="ExternalInput")
    # global ref index per composite low-half: iota[0, j] = j
    iota_d = nc.dram_tensor("iota", [1, NR], u32, kind="ExternalInput")

    # composite top-16 per query: fp16(-d2) in high 16 bits, ref idx in low 14
    outV_d = nc.dram_tensor("outV", [QPC, K], u32, kind="ExternalOutput")

    with tile.TileContext(nc) as tc:
        with (
            tc.tile_pool(name="consts", bufs=1) as cpool,
            tc.tile_pool(name="psum", bufs=2, space="PSUM") as ppool,
            tc.tile_pool(name="merge", bufs=2) as mpool,
        ):
            lhs_t = cpool.tile([66, QPC], f32r)
            # per-chunk lhs slices: chunk 0 only needs its 128 columns
            for c in range(NCHUNK):
                q0, q1 = c * 128, (c + 1) * 128
                nc.sync.dma_start(lhs_t[:, q0:q1], lhs_d.ap()[:, q0:q1])
            rhs_t = cpool.tile([66, NR], f32r)

            # persistent composite staging: one slot per ref-pair-block; low
            # halves carry the global ref index and are written exactly once.
            # Interleave per-pair rhs/iota DMAs (iota on the Act queue) so
            # pair p can start computing as soon as its own slices land.
            stages = []
            for p in range(NPAIR):
                c0, c1 = p * PAIR, (p + 1) * PAIR
                if p == 0:
                    for h in range(PAIR // MMN):
                        s0, s1 = c0 + h * MMN, c0 + (h + 1) * MMN
                        nc.sync.dma_start(rhs_t[:, s0:s1], rhs_d.ap()[:, s0:s1])
                else:
                    nc.sync.dma_start(rhs_t[:, c0:c1], rhs_d.ap()[:, c0:c1])
                st = cpool.tile([128, PAIR], f32, name=f"stage{p}")
                nc.scalar.dma_start(
                    st.bitcast(u32)[:],
                    iota_d.ap()[0:1, c0:c1].partition_broadcast(128),
                )
                stages.append(st)

            # PE pstate warmup: dummy matmuls overlap the input DMA wait so
            # real matmuls start at full clock
            warm = ppool.tile([128, PAIR], f32, tag="ps")
            for _ in range(8):
                nc.tensor.matmul(
                    warm[:, 0:MMN],
                    lhs_t[:, 0:128],
                    rhs_t[:, 0:MMN],
                    start=True,
                    stop=True,
                )

            for rep in range(reps):
              for c in range(NCHUNK):
                cands = mpool.tile([128, NCAND], f32, tag="cands", bufs=2)
                for p in range(NPAIR):
                    ps = ppool.tile([128, PAIR], f32, tag="ps")
                    for h in range(PAIR // MMN):
                        nc.tensor.matmul(
                            ps[:, h * MMN:(h + 1) * MMN],
                            lhs_t[:, c * 128:(c + 1) * 128],
                            rhs_t[:, p * PAIR + h * MMN:p * PAIR + (h + 1) * MMN],
                            start=True,
                            stop=True,
                        )
                    # -d2 as fp16 into composite high halves (strided write)
                    nc.scalar.activation(
                        stages[p].bitcast(f16)[:, 1::2],
                        ps[:],
                        mybir.ActivationFunctionType.Copy,
                    )
                    s = p * 8
                    nc.vector.max(cands[:, s:s + 8], stages[p][:])

                # merge 128 candidate composites -> top-16 (desc = d2 asc)
                v16 = mpool.tile([128, K], f32, tag="v16")
                mr = mpool.tile([128, NCAND], f32, tag="mr")
                nc.vector.max(v16[:, 0:8], cands[:])
                nc.vector.match_replace(mr[:], v16[:, 0:8], cands[:], -1e30)
                nc.vector.max(v16[:, 8:16], mr[:])

                r0, r1 = c * 128, (c + 1) * 128
                nc.sync.dma_start(outV_d.ap()[r0:r1, :], v16.bitcast(u32)[:])

    nc.compile()
    _prog_cache[reps] = nc
    return nc


def kernel(ref: np.ndarray, query: np.ndarray):
    ref = np.asarray(ref, dtype=np.float32)
    query = np.asarray(query, dtype=np.float32)

    # host-side operand prep (layout + norms)
    r2 = np.sum(ref * ref, axis=-1)                      # [B, NR]
    q2 = np.sum(query * query, axis=-1)                  # [B, NQ]
    refT = np.ascontiguousarray(ref.transpose(0, 2, 1))  # [B, D, NR]
    qT = np.ascontiguousarray(query.transpose(0, 2, 1))  # [B, D, NQ]
    iota = np.arange(NR, dtype=np.uint32)[None, :]

    nc = _build_program()

    in_maps = []
    for core in range(NCORES):
        b, h = core // 2, core % 2
        lhs = np.empty((66, QPC), dtype=np.float32)
        lhs[0:D, :] = 2.0 * qT[b][:, h * QPC:(h + 1) * QPC]
        lhs[D, :] = 1.0
        lhs[D + 1, :] = q2[b, h * QPC:(h + 1) * QPC]
        rhs = np.empty((66, NR), dtype=np.float32)
        rhs[0:D, :] = refT[b]
        rhs[D, :] = -r2[b]
        rhs[D + 1, :] = -1.0
        in_maps.append({"lhs": lhs, "rhs": rhs, "iota": iota})

    res = run_bass_kernel_spmd(nc, in_maps, core_ids=list(range(NCORES)))

    Dout = np.empty((B, NQ, K), dtype=np.float32)
    Iout = np.empty((B, NQ, K), dtype=np.int64)
    rows = np.arange(QPC)[:, None]
    for core in range(NCORES):
        b, h = core // 2, core % 2
        comp = res.results[core]["outV"].astype(np.uint32)   # [QPC, K]
        idx = (comp & 0x3FFF).astype(np.int64)
        # exact rescore of the 16 device-selected candidates (fixes
        # quantization-induced order swaps among near-ties)
        qs = query[b, h * QPC:(h + 1) * QPC]                 # [QPC, D]
        cand = ref[b][idx]                                   # [QPC, K, D]
        d2 = np.maximum(0.0, np.sum((cand - qs[:, None, :]) ** 2, axis=-1))
        # sort by (d2, idx): exact ties keep smaller index first
        perm = np.lexsort((idx, d2), axis=1)
        Dout[b, h * QPC:(h + 1) * QPC] = np.sqrt(d2[rows, perm])
        Iout[b, h * QPC:(h + 1) * QPC] = idx[rows, perm]
    return (Dout, Iout)


# revision 10
# speedup vs baseline: 2.0035x; 1.0500x over previous
import sys

sys.path.insert(0, "/opt/trn_rl_repo")

import numpy as np

import concourse.bacc as bacc
import concourse.bass as bass
import concourse.mybir as mybir
import concourse.tile as tile
from concourse.bass_utils import run_bass_kernel_spmd

# Problem shapes (hardcoded per contract)
B = 4
NQ = 2048
NR = 16384
D = 64
K = 16

NCORES = 8
QPC = NQ // 2          # queries per core (each batch split across 2 cores)
NCHUNK = QPC // 128    # query chunks of 128 per core
MMN = 512              # matmul free dim (one PSUM bank of fp32)
PAIR = 2048            # refs per staging tile (4 PSUM banks); top-8 per block
NPAIR = NR // PAIR     # 8
NSLOT = 4              # staging slots (Act->DVE pipeline depth)
NCAND = NPAIR * 8      # 64 candidates per row

_prog_cache = {}


def _build_program(reps: int = 1):
    if reps in _prog_cache:
        return _prog_cache[reps]

    f32 = mybir.dt.float32
    f32r = mybir.dt.float32r
    f16 = mybir.dt.float16
    u32 = mybir.dt.uint32

    nc = bacc.Bacc("TRN2", target_bir_lowering=False, debug=False, num_devices=NCORES)

    # lhsT rows 0..63 = 2*q^T, row 64 = 1.0, row 65 = q2  -> psum = 2qr - r2 - q2 = -d2
    lhs_d = nc.dram_tensor("lhs", [66, QPC], f32r, kind="ExternalInput")
    rhs_d = nc.dram_tensor("rhs", [66, NR], f32r, kind="ExternalInput")

    # composite top-16 per query: fp16(-d2) in high 16 bits, local ref idx in
    # low 11; plus the candidate slot (block) each winner came from
    outV_d = nc.dram_tensor("outV", [QPC, K], u32, kind="ExternalOutput")
    outP_d = nc.dram_tensor("outP", [QPC, K], u32, kind="ExternalOutput")

    with tile.TileContext(nc) as tc:
        with (
            tc.tile_pool(name="consts", bufs=1) as cpool,
            tc.tile_pool(name="psum", bufs=2, space="PSUM") as ppool,
            tc.tile_pool(name="merge", bufs=2) as mpool,
        ):
            lhs_t = cpool.tile([66, QPC], f32r)
            rhs_t = cpool.tile([66, NR], f32r)
            # chunk-0 operands first so compute starts ASAP
            for h in range(PAIR // MMN):
                s0, s1 = h * MMN, (h + 1) * MMN
                nc.sync.dma_start(rhs_t[:, s0:s1], rhs_d.ap()[:, s0:s1])
            for c in range(NCHUNK):
                q0, q1 = c * 128, (c + 1) * 128
                nc.sync.dma_start(lhs_t[:, q0:q1], lhs_d.ap()[:, q0:q1])
            for p in range(1, NPAIR):
                c0, c1 = p * PAIR, (p + 1) * PAIR
                nc.sync.dma_start(rhs_t[:, c0:c1], rhs_d.ap()[:, c0:c1])

            # composite staging slots; low halves = local ref idx (0..PAIR-1),
            # written once by the otherwise-idle gpsimd engine
            stages = []
            for s in range(NSLOT):
                st = cpool.tile([128, PAIR], f32, name=f"stage{s}")
                nc.gpsimd.iota(
                    st.bitcast(u32)[:], pattern=[[1, PAIR]], base=0,
                    channel_multiplier=0,
                )
                stages.append(st)

            # PE pstate warmup: dummy matmuls overlap the input DMA wait so
            # real matmuls start at full clock
            warm = ppool.tile([128, PAIR], f32, tag="ps")
            for _ in range(8):
                nc.tensor.matmul(
                    warm[:, 0:MMN],
                    lhs_t[:, 0:128],
                    rhs_t[:, 0:MMN],
                    start=True,
                    stop=True,
                )

            for rep in range(reps):
              for c in range(NCHUNK):
                cands = mpool.tile([128, NCAND], f32, tag="cands", bufs=2)
                for p in range(NPAIR):
                    ps = ppool.tile([128, PAIR], f32, tag="ps")
                    for h in range(PAIR // MMN):
                        nc.tensor.matmul(
                            ps[:, h * MMN:(h + 1) * MMN],
                            lhs_t[:, c * 128:(c + 1) * 128],
                            rhs_t[:, p * PAIR + h * MMN:p * PAIR + (h + 1) * MMN],
                            start=True,
                            stop=True,
                        )
                    # -d2 as fp16 into composite high halves (strided write)
                    st = stages[p % NSLOT]
                    nc.scalar.activation(
                        st.bitcast(f16)[:, 1::2],
                        ps[:],
                        mybir.ActivationFunctionType.Copy,
                    )
                    s = p * 8
                    nc.vector.max(cands[:, s:s + 8], st[:])

                # merge 64 candidate composites -> top-16 (desc = d2 asc)
                v16 = mpool.tile([128, K], f32, tag="v16")
                p16 = mpool.tile([128, K], u32, tag="p16")
                mr = mpool.tile([128, NCAND], f32, tag="mr")
                nc.vector.max(v16[:, 0:8], cands[:])
                nc.vector.max_index(p16[:, 0:8], v16[:, 0:8], cands[:])
                nc.vector.match_replace(mr[:], v16[:, 0:8], cands[:], -1e30)
                nc.vector.max(v16[:, 8:16], mr[:])
                nc.vector.max_index(p16[:, 8:16], v16[:, 8:16], mr[:])

                r0, r1 = c * 128, (c + 1) * 128
                nc.sync.dma_start(outV_d.ap()[r0:r1, :], v16.bitcast(u32)[:])
                nc.sync.dma_start(outP_d.ap()[r0:r1, :], p16[:])

    nc.compile()
    _prog_cache[reps] = nc
    return nc


def kernel(ref: np.ndarray, query: np.ndarray):
    ref = np.asarray(ref, dtype=np.float32)
    query = np.asarray(query, dtype=np.float32)

    # host-side operand prep (layout + norms)
    r2 = np.sum(ref * ref, axis=-1)                      # [B, NR]
    q2 = np.sum(query * query, axis=-1)                  # [B, NQ]
    refT = np.ascontiguousarray(ref.transpose(0, 2, 1))  # [B, D, NR]
    qT = np.ascontiguousarray(query.transpose(0, 2, 1))  # [B, D, NQ]

    nc = _build_program()

    in_maps = []
    for core in range(NCORES):
        b, h = core // 2, core % 2
        lhs = np.empty((66, QPC), dtype=np.float32)
        lhs[0:D, :] = 2.0 * qT[b][:, h * QPC:(h + 1) * QPC]
        lhs[D, :] = 1.0
        lhs[D + 1, :] = q2[b, h * QPC:(h + 1) * QPC]
        rhs = np.empty((66, NR), dtype=np.float32)
        rhs[0:D, :] = refT[b]
        rhs[D, :] = -r2[b]
        rhs[D + 1, :] = -1.0
        in_maps.append({"lhs": lhs, "rhs": rhs})

    res = run_bass_kernel_spmd(nc, in_maps, core_ids=list(range(NCORES)))

    Dout = np.empty((B, NQ, K), dtype=np.float32)
    Iout = np.empty((B, NQ, K), dtype=np.int64)
    rows = np.arange(QPC)[:, None]
    for core in range(NCORES):
        b, h = core // 2, core % 2
        comp = res.results[core]["outV"].astype(np.uint32)   # [QPC, K]
        slot = res.results[core]["outP"].astype(np.int64)    # [QPC, K]
        idx = (slot >> 3) * PAIR + (comp & 0x7FF).astype(np.int64)
        # exact rescore of the 16 device-selected candidates (fixes
        # quantization-induced order swaps among near-ties)
        qs = query[b, h * QPC:(h + 1) * QPC]                 # [QPC, D]
        cand = ref[b][idx]                                   # [QPC, K, D]
        d2 = np.maximum(0.0, np.sum((cand - qs[:, None, :]) ** 2, axis=-1))
        # sort by (d2, idx): exact ties keep smaller index first
        perm = np.lexsort((idx, d2), axis=1)
        Dout[b, h * QPC:(h + 1) * QPC] = np.sqrt(d2[rows, perm])
        Iout[b, h * QPC:(h + 1) * QPC] = idx[rows, perm]
    return (Dout, Iout)


# revision 11
# speedup vs baseline: 2.0099x; 1.0032x over previous
import sys

sys.path.insert(0, "/opt/trn_rl_repo")

import numpy as np

import concourse.bacc as bacc
import concourse.bass as bass
import concourse.mybir as mybir
import concourse.tile as tile
from concourse.bass_utils import run_bass_kernel_spmd

# Problem shapes (hardcoded per contract)
B = 4
NQ = 2048
NR = 16384
D = 64
K = 16

NCORES = 8
QPC = NQ // 2          # queries per core (each batch split across 2 cores)
NCHUNK = QPC // 128    # query chunks of 128 per core
MMN = 512              # matmul free dim (one PSUM bank of fp32)
PAIR = 2048            # refs per staging tile (4 PSUM banks); top-8 per block
NPAIR = NR // PAIR     # 8
NSLOT = 4              # staging slots (Act->DVE pipeline depth)
NCAND = NPAIR * 8      # 64 candidates per row

_prog_cache = {}


def _build_program(reps: int = 1):
    if reps in _prog_cache:
        return _prog_cache[reps]

    f32 = mybir.dt.float32
    f32r = mybir.dt.float32r
    f16 = mybir.dt.float16
    u32 = mybir.dt.uint32

    nc = bacc.Bacc("TRN2", target_bir_lowering=False, debug=False, num_devices=NCORES)

    # lhsT rows 0..63 = 2*q^T, row 64 = 1.0, row 65 = q2  -> psum = 2qr - r2 - q2 = -d2
    lhs_d = nc.dram_tensor("lhs", [66, QPC], f32r, kind="ExternalInput")
    rhs_d = nc.dram_tensor("rhs", [66, NR], f32r, kind="ExternalInput")

    # composite top-16 per query: fp16(-d2) in high 16 bits, local ref idx in
    # low 11; plus the candidate slot (block) each winner came from
    outV_d = nc.dram_tensor("outV", [QPC, K], u32, kind="ExternalOutput")
    outP_d = nc.dram_tensor("outP", [QPC, K], u32, kind="ExternalOutput")

    with tile.TileContext(nc) as tc:
        with (
            tc.tile_pool(name="consts", bufs=1) as cpool,
            tc.tile_pool(name="psum", bufs=2, space="PSUM") as ppool,
            tc.tile_pool(name="merge", bufs=2) as mpool,
        ):
            lhs_t = cpool.tile([66, QPC], f32r)
            rhs_t = cpool.tile([66, NR], f32r)
            # pair-0 operands first so compute starts ASAP
            nc.sync.dma_start(rhs_t[:, 0:PAIR], rhs_d.ap()[:, 0:PAIR])
            nc.sync.dma_start(lhs_t[:], lhs_d.ap())
            for p in range(1, NPAIR):
                c0, c1 = p * PAIR, (p + 1) * PAIR
                nc.sync.dma_start(rhs_t[:, c0:c1], rhs_d.ap()[:, c0:c1])

            # composite staging slots; low halves = local ref idx (0..PAIR-1),
            # written once by the otherwise-idle gpsimd engine
            stages = []
            for s in range(NSLOT):
                st = cpool.tile([128, PAIR], f32, name=f"stage{s}")
                nc.gpsimd.iota(
                    st.bitcast(u32)[:], pattern=[[1, PAIR]], base=0,
                    channel_multiplier=0,
                )
                stages.append(st)

            # PE pstate warmup: dummy matmuls overlap the input DMA wait so
            # real matmuls start at full clock
            warm = ppool.tile([128, PAIR], f32, tag="ps")
            for _ in range(8):
                nc.tensor.matmul(
                    warm[:, 0:MMN],
                    lhs_t[:, 0:128],
                    rhs_t[:, 0:MMN],
                    start=True,
                    stop=True,
                )

            for rep in range(reps):
              for c in range(NCHUNK):
                cands = mpool.tile([128, NCAND], f32, tag="cands", bufs=2)
                for p in range(NPAIR):
                    ps = ppool.tile([128, PAIR], f32, tag="ps")
                    for h in range(PAIR // MMN):
                        nc.tensor.matmul(
                            ps[:, h * MMN:(h + 1) * MMN],
                            lhs_t[:, c * 128:(c + 1) * 128],
                            rhs_t[:, p * PAIR + h * MMN:p * PAIR + (h + 1) * MMN],
                            start=True,
                            stop=True,
                        )
                    # -d2 as fp16 into composite high halves (strided write)
                    st = stages[p % NSLOT]
                    nc.scalar.activation(
                        st.bitcast(f16)[:, 1::2],
                        ps[:],
                        mybir.ActivationFunctionType.Copy,
                    )
                    s = p * 8
                    nc.vector.max(cands[:, s:s + 8], st[:])

                # merge 64 candidate composites -> top-16 (desc = d2 asc)
                v16 = mpool.tile([128, K], f32, tag="v16")
                p16 = mpool.tile([128, K], u32, tag="p16")
                mr = mpool.tile([128, NCAND], f32, tag="mr")
                nc.vector.max(v16[:, 0:8], cands[:])
                nc.vector.max_index(p16[:, 0:8], v16[:, 0:8], cands[:])
                nc.vector.match_replace(mr[:], v16[:, 0:8], cands[:], -1e30)
                nc.vector.max(v16[:, 8:16], mr[:])
                nc.vector.max_index(p16[:, 8:16], v16[:, 8:16], mr[:])

                r0, r1 = c * 128, (c + 1) * 128
                nc.sync.dma_start(outV_d.ap()[r0:r1, :], v16.bitcast(u32)[:])
                nc.sync.dma_start(outP_d.ap()[r0:r1, :], p16[:])

    nc.compile()
    _prog_cache[reps] = nc
    return nc


def kernel(ref: np.ndarray, query: np.ndarray):
    ref = np.asarray(ref, dtype=np.float32)
    query = np.asarray(query, dtype=np.float32)

    # host-side operand prep (layout + norms)
    r2 = np.sum(ref * ref, axis=-1)                      # [B, NR]
    q2 = np.sum(query * query, axis=-1)                  # [B, NQ]
    refT = np.ascontiguousarray(ref.transpose(0, 2, 1))  # [B, D, NR]
    qT = np.ascontiguousarray(query.transpose(0, 2, 1))  # [B, D, NQ]

    nc = _build_program()

    in_maps = []
    for core in range(NCORES):
        b, h = core // 2, core % 2
        lhs = np.empty((66, QPC), dtype=np.float32)
        lhs[0:D, :] = 2.0 * qT[b][:, h * QPC:(h + 1) * QPC]
        lhs[D, :] = 1.0
        lhs[D + 1, :] = q2[b, h * QPC:(h + 1) * QPC]
        rhs = np.empty((66, NR), dtype=np.float32)
        rhs[0:D, :] = refT[b]
        rhs[D, :] = -r2[b]
        rhs[D + 1, :] = -1.0
        in_maps.append({"lhs": lhs, "rhs": rhs})

    res = run_bass_kernel_spmd(nc, in_maps, core_ids=list(range(NCORES)))

    Dout = np.empty((B, NQ, K), dtype=np.float32)
    Iout = np.empty((B, NQ, K), dtype=np.int64)
    rows = np.arange(QPC)[:, None]
    for core in range(NCORES):
        b, h = core // 2, core % 2
        comp = res.results[core]["outV"].astype(np.uint32)   # [QPC, K]
        slot = res.results[core]["outP"].astype(np.int64)    # [QPC, K]
        idx = (slot >> 3) * PAIR + (comp & 0x7FF).astype(np.int64)
        # exact rescore of the 16 device-selected candidates (fixes
        # quantization-induced order swaps among near-ties)
        qs = query[b, h * QPC:(h + 1) * QPC]                 # [QPC, D]
        cand = ref[b][idx]                                   # [QPC, K, D]
        d2 = np.maximum(0.0, np.sum((cand - qs[:, None, :]) ** 2, axis=-1))
        # sort by (d2, idx): exact ties keep smaller index first
        perm = np.lexsort((idx, d2), axis=1)
        Dout[b, h * QPC:(h + 1) * QPC] = np.sqrt(d2[rows, perm])
        Iout[b, h * QPC:(h + 1) * QPC] = idx[rows, perm]
    return (Dout, Iout)


# revision 12
# speedup vs baseline: 2.1711x; 1.0802x over previous
import sys

sys.path.insert(0, "/opt/trn_rl_repo")

import numpy as np

import concourse.bacc as bacc
import concourse.bass as bass
import concourse.mybir as mybir
import concourse.tile as tile
from concourse.bass_utils import run_bass_kernel_spmd

# Problem shapes (hardcoded per contract)
B = 4
NQ = 2048
NR = 16384
D = 64
K = 16

NCORES = 8
QPC = NQ // 2          # queries per core (each batch split across 2 cores)
NCHUNK = QPC // 128    # query chunks of 128 per core
MMN = 512              # matmul free dim (one PSUM bank of fp32)
PAIR = 2048            # refs per staging tile (4 PSUM banks); top-8 per block
NPAIR = NR // PAIR     # 8
NSLOT = 4              # staging slots (Act->DVE pipeline depth)
NCAND = NPAIR * 8      # 64 candidates per row

_prog_cache = {}


def _build_program(reps: int = 1):
    if reps in _prog_cache:
        return _prog_cache[reps]

    f32 = mybir.dt.float32
    f32r = mybir.dt.float32r
    f16 = mybir.dt.float16
    u32 = mybir.dt.uint32

    nc = bacc.Bacc("TRN2", target_bir_lowering=False, debug=False, num_devices=NCORES)

    # lhsT rows 0..63 = 2*q^T, row 64 = 1.0, row 65 = q2  -> psum = 2qr - r2 - q2 = -d2
    lhs_d = nc.dram_tensor("lhs", [66, QPC], f32r, kind="ExternalInput")
    rhs_d = nc.dram_tensor("rhs", [66, NR], f32r, kind="ExternalInput")

    # 64 candidate composites per query: fp16(-d2) in high 16 bits, local ref
    # idx in low 11; candidate column s comes from ref block s >> 3
    outC_d = nc.dram_tensor("outC", [QPC, NCAND], u32, kind="ExternalOutput")

    with tile.TileContext(nc) as tc:
        with (
            tc.tile_pool(name="consts", bufs=1) as cpool,
            tc.tile_pool(name="psum", bufs=2, space="PSUM") as ppool,
            tc.tile_pool(name="merge", bufs=2) as mpool,
        ):
            lhs_t = cpool.tile([66, QPC], f32r)
            rhs_t = cpool.tile([66, NR], f32r)
            # pair-0 operands first so compute starts ASAP
            nc.sync.dma_start(rhs_t[:, 0:PAIR], rhs_d.ap()[:, 0:PAIR])
            nc.sync.dma_start(lhs_t[:], lhs_d.ap())
            for p in range(1, NPAIR):
                c0, c1 = p * PAIR, (p + 1) * PAIR
                nc.sync.dma_start(rhs_t[:, c0:c1], rhs_d.ap()[:, c0:c1])

            # trigger the activation-table load before real work
            actwarm = cpool.tile([128, 1], f32)
            nc.gpsimd.memset(actwarm[:], 0.0)
            nc.scalar.activation(
                actwarm[:], actwarm[:], mybir.ActivationFunctionType.Copy
            )

            # composite staging slots; low halves = local ref idx (0..PAIR-1),
            # written once by the otherwise-idle gpsimd engine
            stages = []
            for s in range(NSLOT):
                st = cpool.tile([128, PAIR], f32, name=f"stage{s}")
                nc.gpsimd.iota(
                    st.bitcast(u32)[:], pattern=[[1, PAIR]], base=0,
                    channel_multiplier=0,
                )
                stages.append(st)

            for rep in range(reps):
              for c in range(NCHUNK):
                cands = mpool.tile([128, NCAND], u32, tag="cands", bufs=2)
                for p in range(NPAIR):
                    ps = ppool.tile([128, PAIR], f32, tag="ps")
                    for h in range(PAIR // MMN):
                        nc.tensor.matmul(
                            ps[:, h * MMN:(h + 1) * MMN],
                            lhs_t[:, c * 128:(c + 1) * 128],
                            rhs_t[:, p * PAIR + h * MMN:p * PAIR + (h + 1) * MMN],
                            start=True,
                            stop=True,
                        )
                    # -d2 as fp16 into composite high halves (strided write)
                    st = stages[p % NSLOT]
                    nc.scalar.activation(
                        st.bitcast(f16)[:, 1::2],
                        ps[:],
                        mybir.ActivationFunctionType.Copy,
                    )
                    s = p * 8
                    nc.vector.max(cands.bitcast(f32)[:, s:s + 8], st[:])

                r0, r1 = c * 128, (c + 1) * 128
                nc.sync.dma_start(outC_d.ap()[r0:r1, :], cands[:])

    nc.compile()
    _prog_cache[reps] = nc
    return nc


def kernel(ref: np.ndarray, query: np.ndarray):
    ref = np.asarray(ref, dtype=np.float32)
    query = np.asarray(query, dtype=np.float32)

    # host-side operand prep (layout + norms)
    r2 = np.sum(ref * ref, axis=-1)                      # [B, NR]
    q2 = np.sum(query * query, axis=-1)                  # [B, NQ]
    refT = np.ascontiguousarray(ref.transpose(0, 2, 1))  # [B, D, NR]
    qT = np.ascontiguousarray(query.transpose(0, 2, 1))  # [B, D, NQ]

    nc = _build_program()

    in_maps = []
    for core in range(NCORES):
        b, h = core // 2, core % 2
        lhs = np.empty((66, QPC), dtype=np.float32)
        lhs[0:D, :] = 2.0 * qT[b][:, h * QPC:(h + 1) * QPC]
        lhs[D, :] = 1.0
        lhs[D + 1, :] = q2[b, h * QPC:(h + 1) * QPC]
        rhs = np.empty((66, NR), dtype=np.float32)
        rhs[0:D, :] = refT[b]
        rhs[D, :] = -r2[b]
        rhs[D + 1, :] = -1.0
        in_maps.append({"lhs": lhs, "rhs": rhs})

    res = run_bass_kernel_spmd(nc, in_maps, core_ids=list(range(NCORES)))

    # candidate column s -> ref block s >> 3
    base = ((np.arange(NCAND) >> 3) * PAIR).astype(np.int64)[None, :]
    rows = np.arange(QPC)[:, None]
    Dout = np.empty((B, NQ, K), dtype=np.float32)
    Iout = np.empty((B, NQ, K), dtype=np.int64)
    for core in range(NCORES):
        b, h = core // 2, core % 2
        comp = res.results[core]["outC"].astype(np.uint32)   # [QPC, NCAND]
        gidx = base + (comp & 0x7FF).astype(np.int64)        # global ref idx
        # merge: top-16 of 64 by composite order (desc composite = asc d2)
        top = np.argsort(comp.view(np.float32), axis=1, kind="stable")[:, :-K - 1:-1]
        idx = gidx[rows, top]                                # [QPC, K]
        # exact rescore of the 16 selected candidates (fixes quantization-
        # induced order swaps among near-ties)
        qs = query[b, h * QPC:(h + 1) * QPC]                 # [QPC, D]
        cand = ref[b][idx]                                   # [QPC, K, D]
        d2 = np.maximum(0.0, np.sum((cand - qs[:, None, :]) ** 2, axis=-1))
        perm = np.lexsort((idx, d2), axis=1)
        Dout[b, h * QPC:(h + 1) * QPC] = np.sqrt(d2[rows, perm])
        Iout[b, h * QPC:(h + 1) * QPC] = idx[rows, perm]
    return (Dout, Iout)


# revision 13
# speedup vs baseline: 2.1880x; 1.0078x over previous
import sys

sys.path.insert(0, "/opt/trn_rl_repo")

import numpy as np

import concourse.bacc as bacc
import concourse.bass as bass
import concourse.mybir as mybir
import concourse.tile as tile
from concourse.bass_utils import run_bass_kernel_spmd

# Problem shapes (hardcoded per contract)
B = 4
NQ = 2048
NR = 16384
D = 64
K = 16

NCORES = 8
QPC = NQ // 2          # queries per core (each batch split across 2 cores)
NCHUNK = QPC // 128    # query chunks of 128 per core
MMN = 512              # matmul free dim (one PSUM bank of fp32)
PAIR = 2048            # refs per staging tile (4 PSUM banks); top-8 per block
NPAIR = NR // PAIR     # 8
NSLOT = 4              # staging slots (Act->DVE pipeline depth)
NCAND = NPAIR * 8      # 64 candidates per row

_prog_cache = {}


def _build_program(reps: int = 1):
    if reps in _prog_cache:
        return _prog_cache[reps]

    f32 = mybir.dt.float32
    f32r = mybir.dt.float32r
    f16 = mybir.dt.float16
    u32 = mybir.dt.uint32

    nc = bacc.Bacc("TRN2", target_bir_lowering=False, debug=False, num_devices=NCORES)

    # lhsT rows 0..63 = 2*q^T, row 64 = 1.0, row 65 = q2  -> psum = 2qr - r2 - q2 = -d2
    lhs_d = nc.dram_tensor("lhs", [66, QPC], f32r, kind="ExternalInput")
    rhs_d = nc.dram_tensor("rhs", [66, NR], f32r, kind="ExternalInput")

    # 64 candidate composites per query: fp16(-d2) in high 16 bits, local ref
    # idx in low 11; candidate column s comes from ref block s >> 3
    outC_d = nc.dram_tensor("outC", [QPC, NCAND], u32, kind="ExternalOutput")

    with tile.TileContext(nc) as tc:
        with (
            tc.tile_pool(name="consts", bufs=1) as cpool,
            tc.tile_pool(name="psum", bufs=2, space="PSUM") as ppool,
            tc.tile_pool(name="merge", bufs=2) as mpool,
        ):
            lhs_t = cpool.tile([66, QPC], f32r)
            rhs_t = cpool.tile([66, NR], f32r)
            # pair-0 operands first so compute starts ASAP
            nc.sync.dma_start(rhs_t[:, 0:MMN], rhs_d.ap()[:, 0:MMN])
            nc.sync.dma_start(lhs_t[:], lhs_d.ap())
            nc.sync.dma_start(rhs_t[:, MMN:PAIR], rhs_d.ap()[:, MMN:PAIR])
            for p in range(1, NPAIR):
                c0, c1 = p * PAIR, (p + 1) * PAIR
                nc.sync.dma_start(rhs_t[:, c0:c1], rhs_d.ap()[:, c0:c1])

            # trigger the activation-table load before real work
            actwarm = cpool.tile([128, 1], f32)
            nc.gpsimd.memset(actwarm[:], 0.0)
            nc.scalar.activation(
                actwarm[:], actwarm[:], mybir.ActivationFunctionType.Copy
            )

            # composite staging slots; low halves = local ref idx (0..PAIR-1),
            # written once by the otherwise-idle gpsimd engine
            stages = []
            for s in range(NSLOT):
                st = cpool.tile([128, PAIR], f32, name=f"stage{s}")
                nc.gpsimd.iota(
                    st.bitcast(u32)[:], pattern=[[1, PAIR]], base=0,
                    channel_multiplier=0,
                )
                stages.append(st)

            for rep in range(reps):
              for c in range(NCHUNK):
                cands = mpool.tile([128, NCAND], u32, tag="cands", bufs=2)
                for p in range(NPAIR):
                    ps = ppool.tile([128, PAIR], f32, tag="ps")
                    for h in range(PAIR // MMN):
                        nc.tensor.matmul(
                            ps[:, h * MMN:(h + 1) * MMN],
                            lhs_t[:, c * 128:(c + 1) * 128],
                            rhs_t[:, p * PAIR + h * MMN:p * PAIR + (h + 1) * MMN],
                            start=True,
                            stop=True,
                        )
                    # -d2 as fp16 into composite high halves (strided write)
                    st = stages[p % NSLOT]
                    nc.scalar.activation(
                        st.bitcast(f16)[:, 1::2],
                        ps[:],
                        mybir.ActivationFunctionType.Copy,
                    )
                    s = p * 8
                    nc.vector.max(cands.bitcast(f32)[:, s:s + 8], st[:])

                r0, r1 = c * 128, (c + 1) * 128
                nc.sync.dma_start(outC_d.ap()[r0:r1, :], cands[:])

    nc.compile()
    _prog_cache[reps] = nc
    return nc


def kernel(ref: np.ndarray, query: np.ndarray):
    ref = np.asarray(ref, dtype=np.float32)
    query = np.asarray(query, dtype=np.float32)

    # host-side operand prep (layout + norms)
    r2 = np.sum(ref * ref, axis=-1)                      # [B, NR]
    q2 = np.sum(query * query, axis=-1)                  # [B, NQ]
    refT = np.ascontiguousarray(ref.transpose(0, 2, 1))  # [B, D, NR]
    qT = np.ascontiguousarray(query.transpose(0, 2, 1))  # [B, D, NQ]

    nc = _build_program()

    in_maps = []
    for core in range(NCORES):
        b, h = core // 2, core % 2
        lhs = np.empty((66, QPC), dtype=np.float32)
        lhs[0:D, :] = 2.0 * qT[b][:, h * QPC:(h + 1) * QPC]
        lhs[D, :] = 1.0
        lhs[D + 1, :] = q2[b, h * QPC:(h + 1) * QPC]
        rhs = np.empty((66, NR), dtype=np.float32)
        rhs[0:D, :] = refT[b]
        rhs[D, :] = -r2[b]
        rhs[D + 1, :] = -1.0
        in_maps.append({"lhs": lhs, "rhs": rhs})

    res = run_bass_kernel_spmd(nc, in_maps, core_ids=list(range(NCORES)))

    # candidate column s -> ref block s >> 3
    base = ((np.arange(NCAND) >> 3) * PAIR).astype(np.int64)[None, :]
    rows = np.arange(QPC)[:, None]
    Dout = np.empty((B, NQ, K), dtype=np.float32)
    Iout = np.empty((B, NQ, K), dtype=np.int64)
    for core in range(NCORES):
        b, h = core // 2, core % 2
        comp = res.results[core]["outC"].astype(np.uint32)   # [QPC, NCAND]
        gidx = base + (comp & 0x7FF).astype(np.int64)        # global ref idx
        # merge: top-16 of 64 by composite order (desc composite = asc d2)
        top = np.argsort(comp.view(np.float32), axis=1, kind="stable")[:, :-K - 1:-1]
        idx = gidx[rows, top]                                # [QPC, K]
        # exact rescore of the 16 selected candidates (fixes quantization-
        # induced order swaps among near-ties)
        qs = query[b, h * QPC:(h + 1) * QPC]                 # [QPC, D]
        cand = ref[b][idx]                                   # [QPC, K, D]
        d2 = np.maximum(0.0, np.sum((cand - qs[:, None, :]) ** 2, axis=-1))
        perm = np.lexsort((idx, d2), axis=1)
        Dout[b, h * QPC:(h + 1) * QPC] = np.sqrt(d2[rows, perm])
        Iout[b, h * QPC:(h + 1) * QPC] = idx[rows, perm]
    return (Dout, Iout)


# revision 15
# speedup vs baseline: 2.1901x; 1.0010x over previous
import sys

sys.path.insert(0, "/opt/trn_rl_repo")

import numpy as np

import concourse.bacc as bacc
import concourse.bass as bass
import concourse.mybir as mybir
import concourse.tile as tile
from concourse.bass_utils import run_bass_kernel_spmd

# Problem shapes (hardcoded per contract)
B = 4
NQ = 2048
NR = 16384
D = 64
K = 16

NCORES = 8
QPC = NQ // 2          # queries per core (each batch split across 2 cores)
NCHUNK = QPC // 128    # query chunks of 128 per core
MMN = 512              # matmul free dim (one PSUM bank of fp32)
PAIR = 2048            # refs per staging tile (4 PSUM banks); top-8 per block
NPAIR = NR // PAIR     # 8
NSLOT = 4              # staging slots (Act->DVE pipeline depth)
NCAND = NPAIR * 8      # 64 candidates per row

_prog_cache = {}


def _build_program(reps: int = 1):
    if reps in _prog_cache:
        return _prog_cache[reps]

    f32 = mybir.dt.float32
    f32r = mybir.dt.float32r
    f16 = mybir.dt.float16
    u32 = mybir.dt.uint32

    nc = bacc.Bacc("TRN2", target_bir_lowering=False, debug=False, num_devices=NCORES)

    # lhsT rows 0..63 = 2*q^T, row 64 = 1.0, row 65 = q2  -> psum = 2qr - r2 - q2 = -d2
    lhs_d = nc.dram_tensor("lhs", [66, QPC], f32r, kind="ExternalInput")
    rhs_d = nc.dram_tensor("rhs", [66, NR], f32r, kind="ExternalInput")

    # 64 candidate composites per query: fp16(-d2) in high 16 bits, local ref
    # idx in low 11; candidate column s comes from ref block s >> 3
    outC_d = nc.dram_tensor("outC", [QPC, NCAND], u32, kind="ExternalOutput")

    with tile.TileContext(nc) as tc:
        with (
            tc.tile_pool(name="consts", bufs=1) as cpool,
            tc.tile_pool(name="psum", bufs=2, space="PSUM") as ppool,
            tc.tile_pool(name="merge", bufs=2) as mpool,
        ):
            lhs_t = cpool.tile([66, QPC], f32r)
            rhs_t = cpool.tile([66, NR], f32r)
            # pair-0 operands first so compute starts ASAP
            nc.sync.dma_start(rhs_t[:, 0:MMN], rhs_d.ap()[:, 0:MMN])
            nc.sync.dma_start(lhs_t[:], lhs_d.ap())
            nc.sync.dma_start(rhs_t[:, MMN:PAIR], rhs_d.ap()[:, MMN:PAIR])
            for p in range(1, NPAIR):
                c0, c1 = p * PAIR, (p + 1) * PAIR
                nc.sync.dma_start(rhs_t[:, c0:c1], rhs_d.ap()[:, c0:c1])

            # trigger the activation-table load before real work
            actwarm = cpool.tile([128, 1], f32)
            nc.gpsimd.memset(actwarm[:], 0.0)
            nc.scalar.activation(
                actwarm[:], actwarm[:], mybir.ActivationFunctionType.Copy
            )

            # single PE warmup matmul (on the lhs tile) so the first real
            # matmuls run at the ramped clock
            warm = ppool.tile([128, PAIR], f32, tag="ps")
            nc.tensor.matmul(
                warm[:, 0:MMN], lhs_t[:, 0:128], lhs_t[:, 0:MMN],
                start=True, stop=True,
            )

            # composite staging slots; low halves = local ref idx (0..PAIR-1),
            # written once by the otherwise-idle gpsimd engine
            stages = []
            for s in range(NSLOT):
                st = cpool.tile([128, PAIR], f32, name=f"stage{s}")
                nc.gpsimd.iota(
                    st.bitcast(u32)[:], pattern=[[1, PAIR]], base=0,
                    channel_multiplier=0,
                )
                stages.append(st)

            for rep in range(reps):
              for c in range(NCHUNK):
                cands = mpool.tile([128, NCAND], u32, tag="cands", bufs=2)
                for p in range(NPAIR):
                    ps = ppool.tile([128, PAIR], f32, tag="ps")
                    for h in range(PAIR // MMN):
                        nc.tensor.matmul(
                            ps[:, h * MMN:(h + 1) * MMN],
                            lhs_t[:, c * 128:(c + 1) * 128],
                            rhs_t[:, p * PAIR + h * MMN:p * PAIR + (h + 1) * MMN],
                            start=True,
                            stop=True,
                        )
                    # -d2 as fp16 into composite high halves (strided write);
                    # per-bank copies for the pipeline-filling first pair
                    st = stages[p % NSLOT]
                    st16 = st.bitcast(f16)
                    if rep == 0 and c == 0 and p == 0:
                        for h in range(PAIR // MMN):
                            nc.scalar.activation(
                                st16[:, 2 * h * MMN + 1::2][:, 0:MMN],
                                ps[:, h * MMN:(h + 1) * MMN],
                                mybir.ActivationFunctionType.Copy,
                            )
                    else:
                        nc.scalar.activation(
                            st16[:, 1::2],
                            ps[:],
                            mybir.ActivationFunctionType.Copy,
                        )
                    s = p * 8
                    nc.vector.max(cands.bitcast(f32)[:, s:s + 8], st[:])

                r0, r1 = c * 128, (c + 1) * 128
                if c == NCHUNK - 1:
                    # split the final output so the tail DMA is tiny
                    nc.sync.dma_start(
                        outC_d.ap()[r0:r1, 0:NCAND // 2], cands[:, 0:NCAND // 2]
                    )
                    nc.sync.dma_start(
                        outC_d.ap()[r0:r1, NCAND // 2:], cands[:, NCAND // 2:]
                    )
                else:
                    nc.sync.dma_start(outC_d.ap()[r0:r1, :], cands[:])

    nc.compile()
    _prog_cache[reps] = nc
    return nc


def kernel(ref: np.ndarray, query: np.ndarray):
    ref = np.asarray(ref, dtype=np.float32)
    query = np.asarray(query, dtype=np.float32)

    # host-side operand prep (layout + norms)
    r2 = np.sum(ref * ref, axis=-1)                      # [B, NR]
    q2 = np.sum(query * query, axis=-1)                  # [B, NQ]
    refT = np.ascontiguousarray(ref.transpose(0, 2, 1))  # [B, D, NR]
    qT = np.ascontiguousarray(query.transpose(0, 2, 1))  # [B, D, NQ]

    nc = _build_program()

    in_maps = []
    for core in range(NCORES):
        b, h = core // 2, core % 2
        lhs = np.empty((66, QPC), dtype=np.float32)
        lhs[0:D, :] = 2.0 * qT[b][:, h * QPC:(h + 1) * QPC]
        lhs[D, :] = 1.0
        lhs[D + 1, :] = q2[b, h * QPC:(h + 1) * QPC]
        rhs = np.empty((66, NR), dtype=np.float32)
        rhs[0:D, :] = refT[b]
        rhs[D, :] = -r2[b]
        rhs[D + 1, :] = -1.0
        in_maps.append({"lhs": lhs, "rhs": rhs})

    res = run_bass_kernel_spmd(nc, in_maps, core_ids=list(range(NCORES)))

    # candidate column s -> ref block s >> 3
    base = ((np.arange(NCAND) >> 3) * PAIR).astype(np.int64)[None, :]
    rows = np.arange(QPC)[:, None]
    Dout = np.empty((B, NQ, K), dtype=np.float32)
    Iout = np.empty((B, NQ, K), dtype=np.int64)
    for core in range(NCORES):
        b, h = core // 2, core % 2
        comp = res.results[core]["outC"].astype(np.uint32)   # [QPC, NCAND]
        gidx = base + (comp & 0x7FF).astype(np.int64)        # global ref idx
        # merge: top-16 of 64 by composite order (desc composite = asc d2)
        top = np.argsort(comp.view(np.float32), axis=1, kind="stable")[:, :-K - 1:-1]
        idx = gidx[rows, top]                                # [QPC, K]
        # exact rescore of the 16 selected candidates (fixes quantization-
        # induced order swaps among near-ties)
        qs = query[b, h * QPC:(h + 1) * QPC]                 # [QPC, D]
        cand = ref[b][idx]                                   # [QPC, K, D]
        d2 = np.maximum(0.0, np.sum((cand - qs[:, None, :]) ** 2, axis=-1))
        perm = np.lexsort((idx, d2), axis=1)
        Dout[b, h * QPC:(h + 1) * QPC] = np.sqrt(d2[rows, perm])
        Iout[b, h * QPC:(h + 1) * QPC] = idx[rows, perm]
    return (Dout, Iout)
